# revision 1
# baseline (speedup 1.0000x reference)
"""Additive attention kernel for 8 Trainium2 NeuronCores.

Math: scores[b,i,j] = sum_d tanh(q[b,i,d] + k[b,j,d]); out = softmax_j(scores) @ v.

tanh(s) ~= sum_m C[m] sin(W[m] s), separable via
sin(w(q+k)) = sin(wq)cos(wk) + cos(wq)sin(wk) -> bilinear form in
sin/cos features, computed as a rank-768 bf16 PE matmul.

D4V2 config: 4 "direct" frequencies whose sin/cos are evaluated on ScalarE,
plus 2 "derived" frequencies (doubles of direct freqs 2,3) whose features
come from DVE double-angle identities: sin2t = 2 s c, cos2t = 1 - 2 s^2.

Range reduction (AF.Sin only accurate to ~|3.9|):
  t (turns) = (w/2pi) x  via PE pair-diag matmul on host-split hi/lo bf16
  n = round(t + 0.125)   via DVE magic-number round (PSUM -> bf16)
  r = t - n via PE -I matmul accumulate; 2*pi*r in [-3.93, 2.36]
  sin = Sin(2pi r) [ScalarE], cos = Sin(2pi r + pi/2), args <= 3.93.

Softmax without max-subtraction; denominator via ones-column in V;
DVE reciprocal + tensor_scalar normalize. Sharding: B=8 -> 1 batch/core.
"""

import math

import numpy as np
import ml_dtypes

import concourse.bass as bass
import concourse.mybir as mybir
from concourse.bass_utils import run_bass_kernel_spmd

F32 = mybir.dt.float32
BF16 = mybir.dt.bfloat16
AF = mybir.ActivationFunctionType
ALU = mybir.AluOpType

B, L, D = 8, 512, 64
PI = math.pi
TWO_PI = 2.0 * math.pi
MAGIC = 12582912.0  # 1.5 * 2^23
ROFF = 0.125        # residue offset: args to Sin stay within +-2pi*0.625

# D4V2: direct freqs (bf16-exact w/2pi), derived = 2x of direct[2], direct[3]
W_DIRECT0 = [0.2801, 0.8444, 1.4164, 1.9983]
DSUB = [2, 3]


def _bf(x):
    return np.asarray(x).astype(ml_dtypes.bfloat16)


def _fit_consts():
    w2pi = _bf(np.array(W_DIRECT0, np.float64) / TWO_PI).astype(np.float64)
    w_eff = w2pi * TWO_PI
    w_full = np.concatenate([w_eff, 2.0 * w_eff[DSUB]])
    S = 9.8
    sg = np.linspace(-S, S, 4001)
    wts = np.exp(-(sg**2) / 4) + 0.02
    A = np.sin(np.outer(sg, w_full)) * np.sqrt(wts)[:, None]
    lam = 3e-3 * np.sqrt(len(sg))
    Ar = np.vstack([A, lam * np.eye(len(w_full))])
    br = np.concatenate([np.tanh(sg) * np.sqrt(wts), np.zeros(len(w_full))])
    c, *_ = np.linalg.lstsq(Ar, br, rcond=None)
    return w2pi.astype(np.float32), c.astype(np.float32)


W2PI, C = _fit_consts()

_CACHE = {}


def _build():
    nc = bass.Bass()

    dgk_ext = nc.declare_dram_parameter("dgk", [128, 896], BF16, isOutput=False)
    qhl_ext = nc.declare_dram_parameter("qhl", [128, 512], BF16, isOutput=False)
    vh_ext = nc.declare_dram_parameter("vh", [L, 65], BF16, isOutput=False)
    amp_ext = nc.declare_dram_parameter("amp", [128, 7], F32, isOutput=False)
    out_ext = nc.declare_dram_parameter("out", [L, D], F32, isOutput=True)

    from contextlib import ExitStack

    with ExitStack() as ctx:
        e = ctx.enter_context
        DGKQ = e(nc.sbuf_tensor("DGKQ", [128, 1408], BF16))
        AMP = e(nc.sbuf_tensor("AMP", [128, 7], F32))
        VH = e(nc.sbuf_tensor("VH", [128, 4, 65], BF16))
        TMP = e(nc.sbuf_tensor("TMP", [128, 2, 1024], F32))   # round stage 1
        NT = e(nc.sbuf_tensor("NT", [128, 2, 1024], BF16))    # integer n
        # FK chunks: [cosK-p0, cosK-p1, sinK-p0, sinK-p1, dcosK, dsinK]
        FK = e(nc.sbuf_tensor("FK", [128, 6, L], BF16))
        # FQ chunks: [sinQ-p0, sinQ-p1, cosQ-p0, cosQ-p1, dsinQ, dcosQ]
        FQ = e(nc.sbuf_tensor("FQ", [128, 6, L], BF16))
        FQS = e(nc.sbuf_tensor("FQS", [128, 6, L], BF16))     # amp-scaled
        EXPT = e(nc.sbuf_tensor("EXPT", [128, 4, L], BF16))
        RCP = e(nc.sbuf_tensor("RCP", [128, 4], F32))
        OUT = e(nc.sbuf_tensor("OUT", [128, 4, D], F32))
        SCR = e(nc.sbuf_tensor("SCR", [128, L], BF16))
        NRMS = e(nc.sbuf_tensor("NRMS", [128, 2, D], F32))
        WARM = e(nc.sbuf_tensor("WARM", [128, 1], F32))
        PSA = e(nc.psum_tensor([128, 4, L], F32))   # angle banks K0 K1 Q0 Q1
        PSS = e(nc.psum_tensor([128, 4, L], F32))   # scores^T banks
        s_in = e(nc.semaphore("s_in"))
        s_vh = e(nc.semaphore("s_vh"))
        s_t0 = e(nc.semaphore("s_t0"))
        s_tmp = e(nc.semaphore("s_tmp"))
        s_n = e(nc.semaphore("s_n"))
        s_fin = e(nc.semaphore("s_fin"))
        s_act = e(nc.semaphore("s_act"))
        s_der = e(nc.semaphore("s_der"))
        s_fqs = e(nc.semaphore("s_fqs"))
        s_sc = e(nc.semaphore("s_sc"))
        s_exp = e(nc.semaphore("s_exp"))
        s_av = e(nc.semaphore("s_av"))
        s_rcp = e(nc.semaphore("s_rcp"))
        s_nrm = e(nc.semaphore("s_nrm"))
        block = e(nc.Block())

        C0AP = nc.const_aps.aps[(F32, 0.0)]

        def DGs(j):
            return DGKQ[:, j * 128:(j + 1) * 128]
        KHL = DGKQ[:, 384:896]
        QHL = DGKQ[:, 896:1408]

        @block.sync
        def _(sync):
            sync.dma_start(out=DGKQ[:, 0:896], in_=dgk_ext[:]).then_inc(s_in, 16)
            out_r = out_ext.rearrange("(g p) c -> p g c", p=128)
            sync.wait_ge(s_nrm, 4)
            sync.dma_start(out=out_r[:], in_=OUT[:]).then_inc(s_in, 16)
            sync.wait_ge(s_in, 32)   # drain: out DMA landed before teardown

        @block.gpsimd
        def _(gpsimd):
            gpsimd.dma_start(out=AMP[:], in_=amp_ext[:]).then_inc(s_vh, 16)
            gpsimd.dma_start(
                out=VH[:], in_=vh_ext.rearrange("(g p) c -> p g c", p=128)
            ).then_inc(s_vh, 16)

        @block.tensor
        def _(tensor):
            for w in range(6):  # clock-ramp fillers on garbage data
                tensor.matmul(PSS[:, 3, :], DGs(2), KHL,
                              start=True, stop=True, skip_group_check=True)
            # t0: angles in turns into PSA banks (K-p0, K-p1, Q-p0, Q-p1)
            tensor.wait_ge(s_in, 16)  # DG + KHL
            for p in range(2):
                tensor.matmul(PSA[:, p, :], DGs(p), KHL,
                              start=True, stop=False).then_inc(s_t0, 1)
            tensor.wait_ge(s_in, 32)  # + QHL
            for p in range(2):
                tensor.matmul(PSA[:, 2 + p, :], DGs(p), QHL,
                              start=True, stop=False).then_inc(s_t0, 1)
            for w in range(4):  # bridge fillers while rounds run
                tensor.matmul(PSS[:, 3, :], DGs(2), KHL,
                              start=True, stop=True, skip_group_check=True)
            # fin: subtract integer n -> residues
            for g in range(4):
                side, p = g // 2, g % 2
                tensor.wait_ge(s_n, side + 1)
                tensor.matmul(PSA[:, g, :], DGs(2),
                              NT[:, side, p * L:(p + 1) * L],
                              start=False, stop=True).then_inc(s_fin, 1)
                if g == 1:
                    for w in range(2):  # bridge to Q-side round completion
                        tensor.matmul(PSS[:, 3, :], DGs(2), KHL,
                                      start=True, stop=True,
                                      skip_group_check=True)
            for w in range(10):  # p-state warm fillers until scores start
                tensor.matmul(PSS[:, 3, :], DGs(2), KHL,
                              start=True, stop=True, skip_group_check=True)
            # scores: 6 chunks x 4 k-banks, accumulate over chunks
            need_act = {0: 1, 1: 1, 2: 3, 3: 3}
            need_fqs = {0: 1, 1: 2, 2: 3, 3: 4}
            for j in range(4):
                tensor.wait_ge(s_act, need_act[j])
                tensor.wait_ge(s_fqs, need_fqs[j])
                for kb in range(4):
                    tensor.matmul(
                        PSS[:, kb, :],
                        FK[:, j, kb * 128:(kb + 1) * 128],
                        FQS[:, j, :],
                        start=(j == 0), stop=False,
                    )
            tensor.wait_ge(s_der, 2)   # dsinK + dcosK ready
            tensor.wait_ge(s_fqs, 5)
            for kb in range(4):
                tensor.matmul(
                    PSS[:, kb, :], FK[:, 4, kb * 128:(kb + 1) * 128],
                    FQS[:, 4, :], start=False, stop=False,
                )
                tensor.matmul(
                    PSS[:, kb, :], FK[:, 5, kb * 128:(kb + 1) * 128],
                    FQS[:, 5, :], start=False, stop=True,
                ).then_inc(s_sc, 1)
            # AV: 4 k-chunks x 4 q-chunks, out into PSA bank 0 (free)
            tensor.wait_ge(s_vh, 32)
            for kb in range(4):
                tensor.wait_ge(s_exp, kb + 1)
                for ib in range(4):
                    mm = tensor.matmul(
                        PSA[:, ib, 0:65],
                        EXPT[:, kb, ib * 128:(ib + 1) * 128],
                        VH[:, kb, :],
                        start=(kb == 0), stop=(kb == 3),
                    )
                    if kb == 3:
                        mm.then_inc(s_av, 1)

        @block.vector
        def _(vector):
            # rounds stage 2: n_tilde = (tmp - MAGIC) + ROFF -> bf16
            for side in range(2):
                vector.wait_ge(s_tmp, side + 1)
                vector.tensor_scalar(
                    NT[:, side, :], TMP[:, side, :],
                    -MAGIC, ROFF, ALU.add, ALU.add,
                ).then_inc(s_n, 1)
            # amp: base Q sin chunks right after sinQ act
            vector.wait_ge(s_act, 2)
            vector.tensor_scalar_mul(
                FQS[:, 0, :], FQ[:, 0, :], AMP[:, 0:1]).then_inc(s_fqs, 1)
            vector.tensor_scalar_mul(
                FQS[:, 1, :], FQ[:, 1, :], AMP[:, 1:2]).then_inc(s_fqs, 1)
            # derived K features from base K sin/cos (pair1 = chunks 1, 3)
            vector.wait_ge(s_act, 3)
            vector.tensor_tensor(
                FK[:, 5, :], FK[:, 3, :], FK[:, 1, :], ALU.mult
            ).then_inc(s_der, 1)  # dsinK = sK * cK
            vector.tensor_tensor(
                SCR[:], FK[:, 3, :], FK[:, 3, :], ALU.mult,
            )
            vector.tensor_scalar(
                FK[:, 4, :], SCR[:], -2.0, 1.0, ALU.mult, ALU.add,
            ).then_inc(s_der, 1)  # dcosK = 1 - 2 s^2
            vector.wait_ge(s_act, 4)
            vector.tensor_scalar_mul(
                FQS[:, 2, :], FQ[:, 2, :], AMP[:, 0:1]).then_inc(s_fqs, 1)
            vector.tensor_scalar_mul(
                FQS[:, 3, :], FQ[:, 3, :], AMP[:, 1:2]).then_inc(s_fqs, 1)
            # derived Q features (pair1 = chunks 1, 3), then amp both
            vector.tensor_tensor(
                FQ[:, 4, :], FQ[:, 1, :], FQ[:, 3, :], ALU.mult
            ).then_inc(s_der, 1)  # dsinQ = sQ * cQ
            vector.tensor_tensor(
                SCR[:], FQ[:, 1, :], FQ[:, 1, :], ALU.mult,
            )
            vector.tensor_scalar(
                FQS[:, 5, :], SCR[:], AMP[:, 5:6], AMP[:, 6:7],
                ALU.mult, ALU.add,
            ).then_inc(s_der, 1)  # dcosQ with amp fused in
            vector.tensor_scalar_mul(
                FQS[:, 4, :], FQ[:, 4, :], AMP[:, 2:3]).then_inc(s_fqs, 1)
            # reciprocals for softmax denominators; DVE normalizes ib 1, 3
            for ib in range(4):
                vector.wait_ge(s_av, ib + 1)
                vector.reciprocal(RCP[:, ib:ib + 1],
                                  PSA[:, ib, 64:65]).then_inc(s_rcp, 1)
                if ib % 2 == 1:
                    vector.tensor_copy(NRMS[:, ib // 2, :], PSA[:, ib, 0:D])
                    vector.tensor_scalar_mul(
                        OUT[:, ib, :], NRMS[:, ib // 2, :], RCP[:, ib:ib + 1],
                    ).then_inc(s_nrm, 1)

        @block.scalar
        def _(scalar):
            # prewarm trig table during input DMA
            scalar.activation(WARM[:], C0AP, AF.Sin)
            scalar.dma_start(out=DGKQ[:, 896:1408], in_=qhl_ext[:]).then_inc(s_in, 16)
            # rounds stage 1: tmp = t0 + MAGIC (rounds to integer in fp32)
            scalar.wait_ge(s_vh, 16)   # AMP for biases
            for side in range(2):
                scalar.wait_ge(s_t0, 2 * side + 2)
                scalar.activation(
                    TMP[:, side, :], PSA[:, 2 * side:2 * side + 2, :],
                    AF.Identity, bias=AMP[:, 4:5],
                ).then_inc(s_tmp, 1)
            # act order: cosK, sinQ, sinK, cosQ
            scalar.wait_ge(s_fin, 2)
            scalar.activation(FK[:, 0:2, :], PSA[:, 0:2, :], AF.Sin,
                              bias=AMP[:, 3:4], scale=TWO_PI).then_inc(s_act, 1)
            scalar.wait_ge(s_fin, 4)
            scalar.activation(FQ[:, 0:2, :], PSA[:, 2:4, :], AF.Sin,
                              scale=TWO_PI).then_inc(s_act, 1)
            scalar.activation(FK[:, 2:4, :], PSA[:, 0:2, :], AF.Sin,
                              scale=TWO_PI).then_inc(s_act, 1)
            scalar.activation(FQ[:, 2:4, :], PSA[:, 2:4, :], AF.Sin,
                              bias=AMP[:, 3:4], scale=TWO_PI).then_inc(s_act, 1)
            # prewarm exp table while scores run
            scalar.activation(WARM[:], C0AP, AF.Exp)
            for kb in range(4):
                scalar.wait_ge(s_sc, kb + 1)
                scalar.activation(EXPT[:, kb, :], PSS[:, kb, :],
                                  AF.Exp).then_inc(s_exp, 1)
            # normalize even ib on ScalarE (odd ib normalized on DVE)
            for ib in (0, 2):
                scalar.wait_ge(s_rcp, ib + 1)
                scalar.activation(OUT[:, ib, :], PSA[:, ib, 0:D], AF.Identity,
                                  scale=RCP[:, ib:ib + 1]).then_inc(s_nrm, 1)


    return nc


def _get_nc():
    if "nc" not in _CACHE:
        _CACHE["nc"] = _build()
    return _CACHE["nc"]


def _make_consts():
    dg = np.zeros((128, 3, 128), np.float32)
    amp = np.zeros((128, 7), np.float32)
    for j in range(2):
        a, b = 2 * j, 2 * j + 1
        for p in range(64):
            dg[p, j, p] = W2PI[a]
            dg[64 + p, j, p] = W2PI[a]
            dg[p, j, 64 + p] = W2PI[b]
            dg[64 + p, j, 64 + p] = W2PI[b]
        amp[0:64, j] = C[a]
        amp[64:128, j] = C[b]
    for p in range(128):
        dg[p, 2, p] = -1.0
    # sacrificial row: row 64 (lo of dim 0) carries the +0.125 residue offset
    dg[64, 0, :] = 0.125
    dg[64, 1, :] = 0.125
    # derived chunk amp: 2*C (the double-angle identities drop a factor 2)
    amp[0:64, 2] = 2.0 * C[4]
    amp[64:128, 2] = 2.0 * C[5]
    amp[:, 3] = PI / 2
    amp[:, 4] = MAGIC
    amp[:, 5] = -2.0 * amp[:, 2]
    amp[:, 6] = amp[:, 2]
    return _bf(dg), amp


def _make_in_maps(q, k, v):
    dg, amp = _make_consts()
    in_maps = []
    for b in range(B):
        def hilo(x):
            xt = np.ascontiguousarray(x.T.astype(np.float32))      # [64, 512]
            h = _bf(xt)
            lo = _bf(xt - h.astype(np.float32))
            return np.concatenate([h, lo], axis=0)                  # [128, 512]

        qhl = hilo(q[b])
        khl = hilo(k[b])
        qhl[64, :] = 1.0   # sacrificial lo-row of dim 0 -> +0.125 offset
        khl[64, :] = 1.0
        vh = _bf(np.concatenate(
            [v[b].astype(np.float32), np.ones((L, 1), np.float32)], axis=1
        ))
        dgk = np.concatenate([dg.reshape(128, 384), khl], axis=1)
        in_maps.append({"dgk": dgk, "qhl": qhl, "vh": vh, "amp": amp})
    return in_maps


def _run(in_maps, **kw):
    nc = _get_nc()
    return run_bass_kernel_spmd(nc, in_maps, core_ids=list(range(8)), **kw)


def kernel(q: np.ndarray, k: np.ndarray, v: np.ndarray) -> np.ndarray:
    res = _run(_make_in_maps(q, k, v))
    out = np.stack([res.results[b]["out"] for b in range(B)]).astype(np.float32)
    return out

